# revision 1
# baseline (speedup 1.0000x reference)
"""Ragged class-token prepend (packed layout) on 8 Trainium2 NeuronCores.

Op: given x_flat [T, D] (packed rows of B ragged sequences, seg_ids sorted),
produce [T+B, D] where each sequence gains one leading class-token row
(the [1, D] weight).

Strategy (data-parallel over output rows):
  - Output rows are split evenly across 8 cores (R = (T+B)/8 rows each).
  - Each core receives a contiguous R-row window of x_flat (+ the weight
    appended as row R) and small int32 index tensors.
  - Because seg_ids are sorted, output rows are long runs of consecutive
    input rows, broken only at the B class-token insertions.  The device
    program exploits this: indirect-DMA "block gathers" move K=6
    consecutive rows per descriptor (6KB descriptors, near-sequential
    addresses), written back with big contiguous DMAs.  The ~1% of rows
    whose K-run crosses a class-token insertion are patched by a small
    gather+scatter fix-up pass (exact row-level indirect DMAs).
All heavy data movement happens on device; the host only computes index
arrays and slices inputs.

Layout: block b (of NBLK=R/(128*K)) covers output rows [b*128K, (b+1)*128K);
partition p holds the K consecutive rows b*128K + p*K + [0..K).
blk_idx[p, b] = first source row; descriptor = K*D contiguous floats.
"""

import numpy as np

import concourse.bass as bass
import concourse.bacc as bacc
import concourse.mybir as mybir
from concourse.tile import TileContext, add_dep_helper
from concourse.bass_utils import run_bass_kernel_spmd

NCORES = 8
P = 128          # SBUF partitions
K = 6            # consecutive rows per gather descriptor

_program_cache: dict = {}


def build_program(R: int, D: int, k: int = K, nf: int = 8, repeat: int = 1,
                  bufs: int = 8, ntail: int = 0, F: int = 0):
    """SPMD program for one core.

    x_in:    [R+1, D] f32 (row R is the class-token weight row)
    blk_idx: [128, nblk] int32 - descriptor start row per (partition, block)
    tail_src/tail_dst: [128, ntail] int32 - batched F-row fix runs
    fix_src/fix_dst:   [128, nf] int32 - per-row fix entries
    out:     [R, D] f32
    repeat: run the body N times (hardware loop) - for benchmarking only
    """
    rows_pp = R // P            # rows per partition slot (258)
    blocks = [k] * (rows_pp // k)
    if rows_pp % k:
        blocks.append(rows_pp % k)
    nblk = len(blocks)
    # Bacc (not raw Bass): its compile() pass legalizes multi-sem waits
    # (generate_event_semaphores) - walrus rejects >4 waits per instruction
    nc = bacc.Bacc(num_devices=1)
    x_in = nc.dram_tensor("x_in", [R + 1, D], mybir.dt.float32, kind="ExternalInput")
    blk_idx = nc.dram_tensor("blk_idx", [P, nblk], mybir.dt.int32, kind="ExternalInput")
    fix_src = nc.dram_tensor("fix_src", [P, nf], mybir.dt.int32, kind="ExternalInput")
    fix_dst = nc.dram_tensor("fix_dst", [P, nf], mybir.dt.int32, kind="ExternalInput")
    if ntail:
        tail_src = nc.dram_tensor(
            "tail_src", [P, ntail], mybir.dt.int32, kind="ExternalInput")
        tail_dst = nc.dram_tensor(
            "tail_dst", [P, ntail], mybir.dt.int32, kind="ExternalInput")
    out = nc.dram_tensor("out", [R, D], mybir.dt.float32, kind="ExternalOutput")

    with TileContext(nc) as tc:
        with (
            tc.tile_pool(name="idxp", bufs=1) as idxp,
            tc.tile_pool(name="wp", bufs=bufs) as wp,
            tc.tile_pool(name="fp", bufs=4) as fp,
        ):
            bt = idxp.tile([P, nblk], mybir.dt.int32, tag="bt")
            fs = idxp.tile([P, nf], mybir.dt.int32, tag="fs")
            fd = idxp.tile([P, nf], mybir.dt.int32, tag="fd")
            nc.sync.dma_start(bt[:], blk_idx[:])
            nc.sync.dma_start(fs[:], fix_src[:])
            nc.sync.dma_start(fd[:], fix_dst[:])
            if ntail:
                ts = idxp.tile([P, ntail], mybir.dt.int32, tag="ts")
                td = idxp.tile([P, ntail], mybir.dt.int32, tag="td")
                nc.sync.dma_start(ts[:], tail_src[:])
                nc.sync.dma_start(td[:], tail_dst[:])

            def body():
                writes = []
                off = 0
                for b, kb in enumerate(blocks):
                    wt = wp.tile([P, k * D], mybir.dt.float32, tag="wt")
                    # 128 descriptors, each kb*D contiguous floats starting
                    # at row bt[p, b] (dest size sets descriptor length)
                    nc.gpsimd.indirect_dma_start(
                        out=wt[:, : kb * D],
                        out_offset=None,
                        in_=x_in[:],
                        in_offset=bass.IndirectOffsetOnAxis(
                            ap=bt[:, b : b + 1], axis=0
                        ),
                    )
                    w = nc.sync.dma_start(
                        out[off : off + P * kb, :].rearrange(
                            "(p k) c -> p (k c)", p=P
                        ),
                        wt[:, : kb * D],
                    )
                    writes.append(w)
                    off += P * kb

                def scatter_after_writes(sc):
                    for w in writes:
                        add_dep_helper(sc.ins, w.ins, reason="fixup after blocks")

                # batched tail fix-ups: F consecutive rows per descriptor
                for f in range(ntail):
                    tt = fp.tile([P, F * D], mybir.dt.float32, tag="tt")
                    nc.gpsimd.indirect_dma_start(
                        out=tt[:],
                        out_offset=None,
                        in_=x_in[:],
                        in_offset=bass.IndirectOffsetOnAxis(
                            ap=ts[:, f : f + 1], axis=0
                        ),
                    )
                    sc = nc.gpsimd.indirect_dma_start(
                        out=out[:],
                        out_offset=bass.IndirectOffsetOnAxis(
                            ap=td[:, f : f + 1], axis=0
                        ),
                        in_=tt[:],
                        in_offset=None,
                    )
                    scatter_after_writes(sc)
                # per-row fix-ups (class rows, run breaks, clamped edges)
                for f in range(nf):
                    ft = fp.tile([P, D], mybir.dt.float32, tag="ft")
                    nc.gpsimd.indirect_dma_start(
                        out=ft[:],
                        out_offset=None,
                        in_=x_in[:],
                        in_offset=bass.IndirectOffsetOnAxis(
                            ap=fs[:, f : f + 1], axis=0
                        ),
                    )
                    sc = nc.gpsimd.indirect_dma_start(
                        out=out[:],
                        out_offset=bass.IndirectOffsetOnAxis(
                            ap=fd[:, f : f + 1], axis=0
                        ),
                        in_=ft[:],
                        in_offset=None,
                    )
                    scatter_after_writes(sc)

            if repeat == 1:
                body()
            else:
                with tc.For_i(0, repeat, 1):
                    body()
    nc.compile()
    return nc


def shard_inputs(x_flat, weight, seg_ids, num_segments, k: int = K,
                 use_tails: bool = False):
    """Host-side index computation + slicing.

    Returns (in_maps, R, D, nf, ntail, F)."""
    x_flat = np.asarray(x_flat)
    weight = np.asarray(weight, dtype=x_flat.dtype).reshape(1, -1)
    seg_ids = np.asarray(seg_ids)
    T, D = x_flat.shape
    B = int(num_segments)
    N = T + B
    assert N % (NCORES * P) == 0, (T, B)
    R = N // NCORES
    rows_pp = R // P
    blocks = [k] * (rows_pp // k)
    if rows_pp % k:
        blocks.append(rows_pp % k)
    F = k - 1

    # source row (into x_flat) for every output row; -1 marks class rows
    offsets = np.searchsorted(seg_ids, np.arange(B, dtype=seg_ids.dtype))
    src = np.empty(N, dtype=np.int64)
    src[offsets + np.arange(B)] = -1
    src[np.arange(T) + seg_ids + 1] = np.arange(T)

    # per-row (block, partition, pos) for the block layout
    pos_l, end_l, j0_mask = [], [], []
    off = 0
    for kb in blocks:
        jj = np.arange(P * kb)
        pos_l.append(jj % kb)
        end_l.append(off + (jj // kb) * kb + kb - 1)
        off += P * kb
    pos = np.concatenate(pos_l)          # position within descriptor
    dend = np.concatenate(end_l)         # last row of the descriptor

    cores = []
    max_fix, max_tail = 1, 1
    for c in range(NCORES):
        s = src[c * R : (c + 1) * R]
        tok = s >= 0
        if tok.any():
            # token sources within a core are a consecutive ascending range
            w0 = int(s[np.argmax(tok)])
            w0 = max(0, min(w0, T - R))
        else:
            w0 = 0
        lidx = np.where(tok, s - w0, R).astype(np.int64)  # class rows -> R

        # descriptor start rows + expected block-pass value per row
        j0 = np.nonzero(pos == 0)[0]
        start_rows = np.empty(R, np.int64)
        off = 0
        for b, kb in enumerate(blocks):
            blk_rows = slice(off, off + P * kb)
            st = np.minimum(lidx[off + np.arange(P) * kb], R + 1 - kb)
            start_rows[blk_rows] = np.repeat(st, kb)
            off += P * kb
        expected = start_rows + pos
        broken = expected != lidx

        # batched tails: after each class row, F consecutive source rows
        brk = np.nonzero(np.diff(lidx) != 1)[0]  # lidx[i+1] != lidx[i]+1
        cls = np.nonzero(lidx == R)[0]
        t0 = cls + 1
        t0 = t0[(t0 + F <= R)]
        if not use_tails:
            t0 = t0[:0]
        if len(t0):
            # valid iff no break transition inside [t0, t0+F-1)
            nxt = np.searchsorted(brk, t0)
            has_brk = (nxt < len(brk)) & (brk[np.minimum(nxt, len(brk) - 1)] < t0 + F - 1)
            t0 = t0[~has_brk]
        covered = np.zeros(R + F, bool)
        for t in t0:
            covered[t : t + F] = True
        tails = t0
        fix = np.nonzero(broken & ~covered[:R])[0]
        cores.append((w0, lidx, start_rows, tails, fix))
        max_fix = max(max_fix, len(fix))
        max_tail = max(max_tail, len(tails))

    nf = -(-max_fix // P)
    ntail = -(-max_tail // P) if use_tails else 0
    in_maps = []
    for c in range(NCORES):
        w0, lidx, start_rows, tails, fix = cores[c]
        x_in = np.concatenate([x_flat[w0 : w0 + R], weight], axis=0)
        st = start_rows[pos == 0].reshape(len(blocks) if False else -1)
        # [nblk, P] -> [P, nblk]
        nblk = len(blocks)
        stm = np.empty((nblk, P), np.int64)
        off = 0
        for b, kb in enumerate(blocks):
            stm[b] = start_rows[off : off + P * kb : kb]
            off += P * kb
        blk_idx = np.ascontiguousarray(stm.T).astype(np.int32)

        # pad per-row fixes with a benign duplicate: out[0] = x_in[lidx[0]]
        pad = nf * P - len(fix)
        fdst = np.concatenate([fix, np.zeros(pad, np.int64)])
        fsrc = np.concatenate([lidx[fix], np.full(pad, lidx[0])])
        fdst2 = np.ascontiguousarray(fdst.reshape(nf, P).T).astype(np.int32)
        fsrc2 = np.ascontiguousarray(fsrc.reshape(nf, P).T).astype(np.int32)

        if not ntail:
            in_maps.append(
                {"x_in": x_in, "blk_idx": blk_idx,
                 "fix_src": fsrc2, "fix_dst": fdst2})
            continue
        # pad tails with a duplicate of a valid run (or find any clean run)
        if len(tails):
            pt = int(tails[0])
        else:
            good = np.nonzero(np.diff(lidx[: R]) == 1)[0]
            pt = None
            for g in good:
                if g + F <= R and (lidx[g : g + F] == lidx[g] + np.arange(F)).all():
                    pt = int(g)
                    break
            assert pt is not None, "no clean F-run for tail padding"
        padt = ntail * P - len(tails)
        tdst = np.concatenate([tails, np.full(padt, pt, np.int64)])
        tsrc = lidx[tdst]
        tdst2 = np.ascontiguousarray(tdst.reshape(ntail, P).T).astype(np.int32)
        tsrc2 = np.ascontiguousarray(tsrc.reshape(ntail, P).T).astype(np.int32)
        in_maps.append(
            {"x_in": x_in, "blk_idx": blk_idx, "fix_src": fsrc2, "fix_dst": fdst2,
             "tail_src": tsrc2, "tail_dst": tdst2}
        )
    return in_maps, R, D, nf, ntail, F


def kernel_run(inputs: dict, trace: bool = False, repeat: int = 1,
               k: int = K, bufs: int = 8, **spmd_kwargs):
    """Run the full op; returns (output, BassKernelResults)."""
    in_maps, R, D, nf, ntail, F = shard_inputs(**inputs, k=k)
    key = (R, D, k, nf, ntail, F, repeat, bufs)
    if key not in _program_cache:
        _program_cache[key] = build_program(
            R, D, k, nf, repeat=repeat, bufs=bufs, ntail=ntail, F=F)
    nc = _program_cache[key]
    res = run_bass_kernel_spmd(
        nc, in_maps, list(range(NCORES)), trace=trace, **spmd_kwargs
    )
    out = np.concatenate([res.results[i]["out"] for i in range(NCORES)], axis=0)
    return out, res


def kernel(**inputs) -> np.ndarray:
    out, _ = kernel_run(inputs)
    return out

